# revision 65
# baseline (speedup 1.0000x reference)
"""Trainium2 Bass kernel for nn_AttentionBlock (causal attn, softmax over the
QUERY axis (dim=1), post-softmax 1/sqrt(K) scale, residual add).

Sharding: data-parallel over batch B=8, one batch element per NeuronCore.

Orientation trick: the reference softmax normalizes over the *query* index i
for each key column j.  We compute logits transposed, lT[j, i], so that the
normalization axis i is the SBUF free axis: causal mask = one additive-mask
DVE op on the diagonal 512-chunk (into a dedicated PSUM bank — PSUM banks are
single-port, in-place RMW silently corrupts on HW), column max = per-chunk
DVE reduces, exp + denominator-part = one ScalarE activation per chunk.  The
final read contraction takes the E_T strips directly as the matmul stationary
operand:
    read[i, :] = sum_j E_T[j, i] * V'[j, :],  V' = (v + bv) / (denom sqrt(K))

Raw Block style with manual semaphores: the walrus build in this container
supports at most ONE embedded sync-wait per instruction, so all cross-engine
deps are standalone wait_ge instructions with statically computed thresholds.
Same-engine producer->consumer pairs also need explicit waits (engines
pipeline with no drain between instructions).

Matmul dtype: float32r (single-pass fp32, 1 cycle/row at N=512 vs 4 for
exact fp32) for projections + logits; bf16 for the probability-weighted read.
The residual (+x) is added on the host.

Pipeline (per core): x loads are chunked by i so projection groups start
~5us in; the PE runs v, k, q projection groups, then interleaves logits
chunk-groups L(jt) with read groups R(jt-1); DVE does mask/max/recip/scale;
ACT does projection-evacuation-with-bias and exp+rowsum.  Output tiles are
evacuated two iterations late so the DVE chain of jt is never gated on R(jt).
"""

import math
import os
import sys

import numpy as np
import ml_dtypes

for _p in ("/opt/trn_rl_repo", "/root/.axon_site/_ro/trn_rl_repo"):
    if os.path.isdir(_p) and _p not in sys.path:
        sys.path.append(_p)

import concourse.bass as bass
from concourse import mybir
from concourse.bass_utils import run_bass_kernel_spmd

B = 8
D = 512
KS = 512
ND = D // 128  # 4 contraction tiles

F32 = mybir.dt.float32
F32R = mybir.dt.float32r
BF16 = mybir.dt.bfloat16
AOP = mybir.AluOpType
AFT = mybir.ActivationFunctionType

INV_SQRT_K = 1.0 / math.sqrt(KS)
FLT_MIN = float(np.finfo(np.float32).min)

TRACE = False
LAST_RESULTS = None
MMDT = F32R


def _c0(jt):
    return (128 * jt) // 512


def build_nc(T=2048, mmdt=None, debug_dump=False):
    if mmdt is None:
        mmdt = MMDT
    NT = T // 128
    NCH = T // 512
    KQ = ND * NCH  # projection output groups for each of q/k

    nc = bass.Bass("TRN2", target_bir_lowering=False, debug=False, num_devices=B)

    # fp32r reads raw IEEE fp32 bits (keeping ~12 mantissa bits), so the
    # host feeds fp32 bytes straight into f32r tensors — no rounding pass
    xT_d = nc.dram_tensor("xT", [D, T], mmdt, kind="ExternalInput")
    wq_d = nc.dram_tensor("wqT", [D, KS], mmdt, kind="ExternalInput")
    wk_d = nc.dram_tensor("wkT", [D, KS], mmdt, kind="ExternalInput")
    wv_d = nc.dram_tensor("wvT", [D, KS], mmdt, kind="ExternalInput")
    bq_d = nc.dram_tensor("bq", [KS], F32, kind="ExternalInput")
    bk_d = nc.dram_tensor("bk", [KS], F32, kind="ExternalInput")
    bv_d = nc.dram_tensor("bv", [KS], F32, kind="ExternalInput")
    ma_d = nc.dram_tensor("madd", [128, 4, 512], BF16, kind="ExternalInput")
    out_d = nc.dram_tensor("out", [T, KS], F32, kind="ExternalOutput")
    if debug_dump:
        de_d = nc.dram_tensor("dbg_e", [128, NT, T], BF16, kind="ExternalOutput")
        dv_d = nc.dram_tensor("dbg_vp", [128, NT, KS], BF16, kind="ExternalOutput")
        dq_d = nc.dram_tensor("dbg_q", [128, ND, T], F32, kind="ExternalOutput")
        dk_d = nc.dram_tensor("dbg_k", [128, ND, T], F32, kind="ExternalOutput")

    # ---- SBUF ----
    xTr = nc.alloc_sbuf_tensor("xTr", [128, ND, T], mmdt)
    wvr = nc.alloc_sbuf_tensor("wvr", [128, ND, KS], mmdt)
    wkr = nc.alloc_sbuf_tensor("wkr", [128, ND, KS], mmdt)
    wqr = nc.alloc_sbuf_tensor("wqr", [128, ND, KS], mmdt)
    kT = nc.alloc_sbuf_tensor("kT", [128, ND, T], mmdt)
    qT = nc.alloc_sbuf_tensor("qT", [128, ND, T], mmdt)
    v_sb = nc.alloc_sbuf_tensor("v_sb", [128, NT, KS], BF16)
    vp_sb = nc.alloc_sbuf_tensor("vp_sb", [128, NT, KS], BF16)
    e_sbs = [
        nc.alloc_sbuf_tensor(f"e{jt}", [128, T - 512 * _c0(jt)], BF16)
        for jt in range(NT)
    ]
    outst = nc.alloc_sbuf_tensor("outst", [128, 2, KS], F32)
    lm = nc.alloc_sbuf_tensor("lm", [128, 2, 512], F32)
    madd = nc.alloc_sbuf_tensor("madd_sb", [128, 4, 512], BF16)
    bqc = nc.alloc_sbuf_tensor("bqc", [128, ND], F32)
    bkc = nc.alloc_sbuf_tensor("bkc", [128, ND], F32)
    bvb = nc.alloc_sbuf_tensor("bvb", [128, KS], F32)
    macc = nc.alloc_sbuf_tensor("macc", [128, 2, NCH], F32)
    negmax = nc.alloc_sbuf_tensor("negmax", [128, 2], F32)
    dparts = nc.alloc_sbuf_tensor("dparts", [128, 2, NCH], F32)
    denom = nc.alloc_sbuf_tensor("denom", [128, 2], F32)
    dscr = nc.alloc_sbuf_tensor("dscr", [128, 4], F32)
    rec = nc.alloc_sbuf_tensor("rec", [128, 2], F32)

    # ---- PSUM: 8 banks of [128, 512] fp32 ----
    ps = [nc.alloc_psum_tensor(f"ps{i}", [128, 512], F32) for i in range(8)]
    # phase 1: v -> ps[0..1], k/q -> ps[2..5]
    # phase 2: logits chunks -> ps[g % 6]; masked diag goes to the SBUF
    # strip lm[:, jt%2] (PSUM banks are single-port: no in-place RMW);
    # read -> ps[6 + jt % 2]

    # ================= static op-index tables =================
    def _la2(j):
        return 2

    # ---- DVE plan (sDV counts every DVE op) ----
    MADD, REDL, NMX, RECIP = {}, {}, {}, {}
    dv = 0
    VCP = {}
    for jt in range(NT):
        dv += 1
        VCP[jt] = dv
    for jt in range(NT):
        nch = NCH - _c0(jt)
        dv += 1
        MADD[jt] = dv
        if nch == 1:
            dv += 1
            NMX[jt] = dv
        else:
            dv += nch
            REDL[jt] = dv
            dv += 1
            NMX[jt] = dv
        dv += 1
        RECIP[jt] = dv

    # ---- ACT plan (sAC): kq copies, then per jt block:
    #      exps, rsum (nch>1), vp, due outcopies ----
    EXP, RSUMA, VPA, OCPA = {}, {}, {}, {}
    oc_due = {}
    for j in range(NT):
        oc_due.setdefault(j + _la2(j), []).append(j)
    ac = 2 * KQ
    for jt in range(NT):
        nch = NCH - _c0(jt)
        for j in oc_due.get(jt, []):
            ac += 1
            OCPA[j] = ac
        for ic in list(range(_c0(jt) + 1, NCH)) + [_c0(jt)]:
            ac += 1
            EXP[(jt, ic)] = ac
        if nch > 1:
            ac += 1
            RSUMA[jt] = ac
        ac += 1
        VPA[jt] = ac
    for j in range(NT):
        if j + _la2(j) >= NT:
            ac += 1
            OCPA[j] = ac
    EXP_END = {jt: EXP[(jt, _c0(jt))] for jt in range(NT)}  # diag emitted last

    # ---- PE plan (sPE counts GROUPS) ----
    # phase 1 emitted per x-chunk batch: v(4ic..4ic+3), k(:,ic), q(:,ic)
    VG, KG, QG = {}, {}, {}
    KQSEQ = {}  # interleaved k/q copy sequence index (ACT order)
    pe = 0
    kqseq = 0
    p1_order = []
    for ic in range(NCH):
        for jt in range(4 * ic, min(4 * (ic + 1), NT)):
            pe += 1
            VG[jt] = pe
            p1_order.append(("v", jt))
        for kt in range(ND):
            pe += 1
            KG[(kt, ic)] = pe
            kqseq += 1
            KQSEQ[("k", kt, ic)] = kqseq
            p1_order.append(("k", kt, ic))
        for kt in range(ND):
            pe += 1
            QG[(kt, ic)] = pe
            kqseq += 1
            KQSEQ[("q", kt, ic)] = kqseq
            p1_order.append(("q", kt, ic))
    assert pe == NT + 2 * KQ
    LG, RG = {}, {}

    _la = _la2
    fused_order = []
    for m in range(NT):
        fused_order.append(("L", m))
        for j in range(NT):
            if j + _la(j) == m:
                fused_order.append(("R", j))
    for j in range(NT):
        if j + _la(j) >= NT:
            fused_order.append(("R", j))
    gctr = 0
    chunk_of_g = {}
    for kind, jt in fused_order:
        if kind == "L":
            for ic in range(_c0(jt), NCH):
                pe += 1
                LG[(jt, ic)] = pe
                chunk_of_g[gctr] = (jt, ic)
                gctr += 1
        else:
            pe += 1
            RG[jt] = pe
    CUMCH = {}
    cc = 0
    for jt in range(NT):
        cc += NCH - _c0(jt)
        CUMCH[jt] = cc

    with (
        nc.semaphore("sLv") as sLv,
        nc.semaphore("sLk") as sLk,
        nc.semaphore("sLq") as sLq,
        nc.semaphore("sLc") as sLc,
        nc.semaphore("sLx0") as sLx0,
        nc.semaphore("sLx1") as sLx1,
        nc.semaphore("sLx2") as sLx2,
        nc.semaphore("sLx3") as sLx3,
        nc.semaphore("sPE") as sPE,
        nc.semaphore("sDV") as sDV,
        nc.semaphore("sAC") as sAC,
        nc.semaphore("sST") as sST,
        nc.Block() as block,
    ):
        sLxs = [sLx0, sLx1, sLx2, sLx3]

        @block.sync
        def _(sp):
            def ldx(ic):
                sp.dma_start(
                    out=xTr[:, :, 512 * ic : 512 * (ic + 1)],
                    in_=xT_d.ap()[:, 512 * ic : 512 * (ic + 1)].rearrange(
                        "(t p) i -> p t i", p=128
                    ),
                ).then_inc(sLxs[ic], 16)

            # load order tracks first use: wv, x0, wk, wq, consts, x1..x3
            sp.dma_start(
                out=wvr[:, :, :],
                in_=wv_d.ap().rearrange("(t p) k -> p t k", p=128),
            ).then_inc(sLv, 16)
            sp.dma_start(
                out=wkr[:, :, :],
                in_=wk_d.ap().rearrange("(t p) k -> p t k", p=128),
            ).then_inc(sLk, 16)
            ldx(0)
            sp.dma_start(
                out=wqr[:, :, :],
                in_=wq_d.ap().rearrange("(t p) k -> p t k", p=128),
            ).then_inc(sLq, 16)
            with nc.allow_non_contiguous_dma(reason="16B/partition bias loads"):
                sp.dma_start(
                    out=bqc[:, :], in_=bq_d.ap().rearrange("(t p) -> p t", p=128)
                ).then_inc(sLc, 16)
                sp.dma_start(
                    out=bkc[:, :], in_=bk_d.ap().rearrange("(t p) -> p t", p=128)
                ).then_inc(sLc, 16)
            bv_ap = bv_d.ap()
            bv_bcast = bass.AP(
                tensor=bv_ap.tensor, offset=bv_ap.offset, ap=[[0, 128]] + list(bv_ap.ap)
            )
            sp.dma_start(out=bvb[:, :], in_=bv_bcast).then_inc(sLc, 16)
            sp.dma_start(out=madd[:, :, :], in_=ma_d.ap()).then_inc(sLc, 16)
            for ic in range(1, NCH):
                ldx(ic)
            # stores
            out_ap = out_d.ap()
            for jt in range(NT):
                sp.wait_ge(sAC, OCPA[jt])
                sp.dma_start(
                    out=out_ap[128 * jt : 128 * (jt + 1), :],
                    in_=outst[:, jt % 2, :],
                ).then_inc(sST, 16)
            if debug_dump:
                sp.wait_ge(sAC, EXP_END[NT - 1])
                for jt in range(NT):
                    i0 = 512 * _c0(jt)
                    sp.dma_start(
                        out=de_d.ap()[:, jt, i0:T], in_=e_sbs[jt][:, :]
                    ).then_inc(sST, 16)
                sp.wait_ge(sAC, OCPA[NT - 1])
                sp.dma_start(out=dv_d.ap(), in_=vp_sb[:, :, :]).then_inc(sST, 16)
                sp.dma_start(out=dq_d.ap(), in_=qT[:, :, :].bitcast(F32)).then_inc(
                    sST, 16
                )
                sp.dma_start(out=dk_d.ap(), in_=kT[:, :, :].bitcast(F32)).then_inc(
                    sST, 16
                )
                sp.wait_ge(sST, 16 * (NT + NT + 3))
            else:
                sp.wait_ge(sST, 16 * NT)

        @block.vector
        def _(ve):
            ndv = 0  # running op index, asserted against the plan

            def inc(x):
                nonlocal ndv
                ndv += 1
                x.then_inc(sDV, 1)

            # v strips: psum + bv -> bf16
            ve.wait_ge(sLc, 64)
            for jt in range(NT):
                ve.wait_ge(sPE, VG[jt])
                inc(
                    ve.tensor_tensor(
                        out=v_sb[:, jt, :], in0=ps[jt % 2][:, :],
                        in1=bvb[:, :], op=AOP.add,
                    )
                )
                assert ndv == VCP[jt]

            # fused loop
            gbank = {}
            g = 0
            for jt in range(NT):
                for ic in range(_c0(jt), NCH):
                    gbank[(jt, ic)] = ps[g % 6]
                    g += 1
            for jt in range(NT):
                c0 = _c0(jt)
                off = jt - 4 * c0
                nch = NCH - c0
                if jt >= 2:
                    # negmax/macc[jt%2] reuse: exp(jt-2) must have read them
                    ve.wait_ge(sAC, EXP_END[jt - 2])
                # causal mask on the diagonal chunk (additive -3e38) into
                # the SBUF strip lm[jt%2] — NOT in place (single-port PSUM);
                # lm[jt%2] reuse is covered by the EXP_END[jt-2] wait above
                ve.wait_ge(sPE, LG[(jt, c0)])
                inc(
                    ve.tensor_tensor(
                        out=lm[:, jt % 2, :], in0=gbank[(jt, c0)][:, :],
                        in1=madd[:, off, :], op=AOP.add,
                    )
                )
                assert ndv == MADD[jt]
                if nch == 1:
                    ve.wait_ge(sDV, MADD[jt])  # same-engine RAW fence
                    inc(
                        ve.reduce_max(
                            negmax[:, jt % 2 : jt % 2 + 1], lm[:, jt % 2, :],
                            mybir.AxisListType.X, negate=True,
                        )
                    )
                    assert ndv == NMX[jt]
                else:
                    # per-chunk column maxes; diagonal chunk reduced LAST and
                    # fenced against the mask-add that wrote its bank
                    for ic in list(range(c0 + 1, NCH)) + [c0]:
                        if ic != c0:
                            ve.wait_ge(sPE, LG[(jt, ic)])
                            src_bank = gbank[(jt, ic)]
                        else:
                            ve.wait_ge(sDV, MADD[jt])
                            src_bank = lm[:, jt % 2 : jt % 2 + 1, :]
                        inc(
                            ve.reduce_max(
                                macc[:, jt % 2, ic : ic + 1], src_bank[:, :],
                                mybir.AxisListType.X,
                            )
                        )
                    assert ndv == REDL[jt]
                    ve.wait_ge(sDV, REDL[jt])  # same-engine RAW fence
                    inc(
                        ve.reduce_max(
                            negmax[:, jt % 2 : jt % 2 + 1],
                            macc[:, jt % 2, c0:NCH],
                            mybir.AxisListType.X, negate=True,
                        )
                    )
                    assert ndv == NMX[jt]
                if nch == 1:
                    ve.wait_ge(sAC, EXP_END[jt])
                    src = dparts[:, jt % 2, c0 : c0 + 1]
                else:
                    # ACT's rsum accumulated the denominator; its index also
                    # covers the rec[jt%2] reuse (vp(jt-2) is ACT, earlier)
                    ve.wait_ge(sAC, RSUMA[jt])
                    src = denom[:, jt % 2 : jt % 2 + 1]
                inc(ve.reciprocal(rec[:, jt % 2 : jt % 2 + 1], src))
                assert ndv == RECIP[jt]

        @block.scalar
        def _(ac_):
            ac_.wait_ge(sLc, 64)
            for ic in range(NCH):
                for wsel, g_tab, bias in ((0, KG, bkc), (1, QG, bqc)):
                    dst = kT if wsel == 0 else qT
                    for kt in range(ND):
                        seq = KQSEQ[("k" if wsel == 0 else "q", kt, ic)]
                        ac_.wait_ge(sPE, g_tab[(kt, ic)])
                        bank = ps[2 + ((seq - 1) % 4)][:, :]
                        ac_.activation(
                            out=dst[:, kt, 512 * ic : 512 * (ic + 1)],
                            in_=bank,
                            func=AFT.Identity,
                            bias=bias[:, kt : kt + 1],
                            scale=1.0,
                        ).then_inc(sAC, 1)
            # per-jt: exp strips, denominator sum, V' scale, due outcopies
            oc_due2 = {}
            for j in range(NT):
                oc_due2.setdefault(j + _la2(j), []).append(j)

            def outcopy(j):
                ac_.wait_ge(sPE, RG[j])
                if j >= 2:
                    # all stores issued so far must be complete (HWDGE queues
                    # finish out of order; partial counts can't pin which)
                    ac_.wait_ge(sST, 16 * j)
                ac_.activation(
                    out=outst[:, j % 2, :], in_=ps[6 + j % 2][:, :], func=AFT.Copy
                ).then_inc(sAC, 1)

            gbank2 = {}
            g = 0
            for jt in range(NT):
                for ic in range(_c0(jt), NCH):
                    gbank2[(jt, ic)] = ps[g % 6]
                    g += 1
            for jt in range(NT):
                c0 = _c0(jt)
                nch = NCH - c0
                for j in oc_due2.get(jt, []):
                    outcopy(j)
                first = True
                for ic in list(range(c0 + 1, NCH)) + [c0]:
                    bank = lm[:, jt % 2 : jt % 2 + 1, :] if ic == c0 else gbank2[(jt, ic)]
                    if first:
                        ac_.wait_ge(sDV, NMX[jt])
                        if jt >= 2 and (NCH - _c0(jt - 2)) == 1:
                            # dparts[jt%2] was read by DVE recip(jt-2)
                            ac_.wait_ge(sDV, RECIP[jt - 2])
                        first = False
                    ac_.activation(
                        out=e_sbs[jt][:, 512 * (ic - c0) : 512 * (ic - c0 + 1)],
                        in_=bank[:, :],
                        func=AFT.Exp,
                        bias=negmax[:, jt % 2 : jt % 2 + 1],
                        scale=1.0,
                        accum_out=dparts[:, jt % 2, ic : ic + 1],
                    ).then_inc(sAC, 1)
                if nch > 1:
                    # denominator = sum of the per-chunk exp sums (Copy+accum)
                    ac_.wait_ge(sAC, EXP_END[jt])  # same-engine RAW fence
                    ac_.activation(
                        out=dscr[:, 0:nch],
                        in_=dparts[:, jt % 2, c0:NCH],
                        func=AFT.Copy,
                        accum_out=denom[:, jt % 2 : jt % 2 + 1],
                    ).then_inc(sAC, 1)
                # V' = v * (1/denom); 1/sqrt(K) is folded into Wv on the host
                ac_.wait_ge(sDV, RECIP[jt])
                ac_.activation(
                    out=vp_sb[:, jt, :], in_=v_sb[:, jt, :], func=AFT.Copy,
                    scale=rec[:, jt % 2 : jt % 2 + 1],
                ).then_inc(sAC, 1)
            for j in range(NT):
                if j + _la2(j) >= NT:
                    outcopy(j)

        @block.tensor
        def _(te):
            # phase 1 per x-chunk batch: v(4ic..4ic+3), k(:,ic), q(:,ic)
            waited = set()

            def ldwait(sem):
                if sem not in waited:
                    te.wait_ge(sem, 16)
                    waited.add(sem)

            for item in p1_order:
                if item[0] == "v":
                    jt = item[1]
                    ldwait(sLv)
                    ldwait(sLxs[jt // 4])
                    if jt >= 2:
                        te.wait_ge(sDV, VCP[jt - 2])
                    for dt_ in range(ND):
                        mm = te.matmul(
                            ps[jt % 2][:, :],
                            lhsT=xTr[:, dt_, 128 * jt : 128 * (jt + 1)],
                            rhs=wvr[:, dt_, :],
                            start=(dt_ == 0),
                            stop=(dt_ == ND - 1),
                        )
                        if dt_ == ND - 1:
                            mm.then_inc(sPE, 1)
                else:
                    kind, kt, ic = item
                    wsb = wkr if kind == "k" else wqr
                    ldwait(sLk if kind == "k" else sLq)
                    ldwait(sLxs[ic])
                    seq = KQSEQ[(kind, kt, ic)]
                    if seq > 4:
                        te.wait_ge(sAC, seq - 4)
                    for dt_ in range(ND):
                        mm = te.matmul(
                            ps[2 + ((seq - 1) % 4)][:, :],
                            lhsT=wsb[:, dt_, 128 * kt : 128 * (kt + 1)],
                            rhs=xTr[:, dt_, 512 * ic : 512 * (ic + 1)],
                            start=(dt_ == 0),
                            stop=(dt_ == ND - 1),
                        )
                        if dt_ == ND - 1:
                            mm.then_inc(sPE, 1)
            # fused: logits chunk groups + read groups
            g = 0
            for kind, jt in fused_order:
                c0 = _c0(jt)
                if kind == "L":
                    for ic in range(c0, NCH):
                        need_ac = KQSEQ[("q", ND - 1, ic)]  # q copies thru ic
                        need_dv = None
                        if g >= 6:
                            pj, pic = chunk_of_g[g - 6]
                            if pic == _c0(pj):
                                # diag bank is released by its mask-add
                                need_dv = MADD[pj]
                            else:
                                need_ac = max(need_ac, EXP[(pj, pic)])
                        elif g % 6 >= 2:
                            need_ac = max(need_ac, 2 * KQ)
                        te.wait_ge(sAC, need_ac)
                        if need_dv is not None:
                            te.wait_ge(sDV, need_dv)
                        if g < 2:
                            # banks 0,1 last used by the v-copy stream (DVE)
                            te.wait_ge(sDV, VCP[NT - 2 + g])
                        bank = ps[g % 6]
                        g += 1
                        for kt in range(ND):
                            mm = te.matmul(
                                bank[:, :],
                                lhsT=kT[:, kt, 128 * jt : 128 * (jt + 1)],
                                rhs=qT[:, kt, 512 * ic : 512 * (ic + 1)],
                                start=(kt == 0),
                                stop=(kt == ND - 1),
                            )
                            if kt == ND - 1:
                                mm.then_inc(sPE, 1)
                else:
                    # early MMs only need strips/vp of j2 <= jt-1; the final
                    # MM (j2 == jt) additionally needs this jt's E and V'
                    need = VPA[jt - 1] if jt >= 1 else 0
                    if jt >= 2:
                        need = max(need, OCPA[jt - 2])
                    if need:
                        te.wait_ge(sAC, need)
                    for j2 in range(jt + 1):
                        if j2 == jt:
                            te.wait_ge(sAC, VPA[jt])
                        i0 = 512 * _c0(j2)
                        mm = te.matmul(
                            ps[6 + jt % 2][:, :],
                            lhsT=e_sbs[j2][:, 128 * jt - i0 : 128 * (jt + 1) - i0],
                            rhs=vp_sb[:, j2, :],
                            start=(j2 == 0),
                            stop=(j2 == jt),
                        )
                        if j2 == jt:
                            mm.then_inc(sPE, 1)

    nc.finalize()
    return nc


def _host_inputs(xb, wqT, wkT, wvT, bq, bk, bv, T):
    # additive causal mask for the diagonal chunk, per offset class o:
    # madd[p, o, x] = 0 where x >= 128*o + p else -3e38
    p = np.arange(128, dtype=np.float32)
    xx = np.arange(512, dtype=np.float32)[None, None, :]
    thr = (p[:, None, None] + 128.0 * np.arange(4, dtype=np.float32)[None, :, None])
    madd = np.where(xx >= thr, 0.0, -3.0e38).astype(ml_dtypes.bfloat16)
    return dict(
        xT=np.ascontiguousarray(xb.T),
        wqT=wqT,
        wkT=wkT,
        wvT=wvT,
        bq=bq,
        bk=bk,
        bv=bv,
        madd=np.ascontiguousarray(madd),
    )


def kernel(x, Wk, bk, Wq, bq, Wv, bv):
    global LAST_RESULTS
    T = 2048
    x = np.ascontiguousarray(np.asarray(x, dtype=np.float32))
    Wk = np.asarray(Wk, dtype=np.float32)
    Wq = np.asarray(Wq, dtype=np.float32)
    Wv = np.asarray(Wv, dtype=np.float32)
    bk = np.ascontiguousarray(np.asarray(bk, dtype=np.float32))
    bq = np.ascontiguousarray(np.asarray(bq, dtype=np.float32))
    bv = np.ascontiguousarray(np.asarray(bv, dtype=np.float32))

    wqT = np.ascontiguousarray(Wq.T)
    wkT = np.ascontiguousarray(Wk.T)
    # fold the post-softmax 1/sqrt(K) into the V projection
    wvT = np.ascontiguousarray(Wv.T * np.float32(INV_SQRT_K))
    bv = np.ascontiguousarray(bv * np.float32(INV_SQRT_K))

    nc = build_nc(T, MMDT)
    in_maps = [_host_inputs(x[b], wqT, wkT, wvT, bq, bk, bv, T) for b in range(B)]
    res = run_bass_kernel_spmd(nc, in_maps, list(range(B)), trace=TRACE)
    LAST_RESULTS = res
    read = np.stack([np.asarray(res.results[b]["out"]) for b in range(B)], axis=0)
    # residual add on host (elementwise, ~0.1% of the FLOPs)
    return (x + read).astype(np.float32)
